# revision 1
# baseline (speedup 1.0000x reference)
"""CrossFeatureAttention TRN2 kernel.

Full inputs -> full output. Sharding: data-parallel over (batch b, half of N1)
across 8 cores; each core computes out[b, h*2048:(h+1)*2048, :].

Math (per core, x1 slice q=2048 rows, x2[b] k=4096 rows, C=512):
    Q  = x1 @ Wq^T + bq
    K  = x2 @ Wk^T + bk
    V  = x2 @ Wv^T + bv
    P  = softmax(Q K^T / sqrt(C))          (no max subtraction; scores are small)
    out = (Q + P V) @ Wo^T + bo
        = x1 @ (Wo Wq)^T + (P V) @ Wo^T + (Wo bq + bo)     <- residual folded

The x1 @ (Wo Wq)^T term carries almost all of the output magnitude and runs in
fp32r; the attention path runs in bf16.  Attention is computed transposed
(S^T[k,q] = sum_c K^T[c,k] Q^T[c,q]) so exp(S^T) is already in the layout the
A^T matmul needs, and row sums come from a ones-matmul over partitions.
"""

import os
import sys

import numpy as np

for _p in ("/root/.axon_site", "/root/.axon_site/_ro/trn_rl_repo",
           "/root/.axon_site/_ro/pypackages"):
    if _p not in sys.path and os.path.isdir(_p):
        sys.path.append(_p)

import ml_dtypes

import concourse.bacc as bacc
import concourse.mybir as mybir
import concourse.tile as tile
from concourse import bass_isa, library_config, masks
from concourse.bass_utils import run_bass_kernel_spmd

F32 = mybir.dt.float32
F32R = mybir.dt.float32r
BF16 = mybir.dt.bfloat16
AF = mybir.ActivationFunctionType

B, N1, N2, C = 4, 4096, 4096, 512
NCORES = 8
QROWS = N1 * B // NCORES          # 2048 q rows per core
QC = 512                          # q-chunk (columns of S^T tiles)
NQC = QROWS // QC                 # 4 chunks
KT = N2 // 128                    # 32 k-tiles
CCH = C // 128                    # 4 contraction chunks
SCALE = 1.0 / float(np.sqrt(C))

_BUILT = None


def build():
    nc = bacc.Bacc(None, target_bir_lowering=False, debug=False)

    x1f_d = nc.dram_tensor("x1f", [QROWS, C], F32, kind="ExternalInput")
    x1b_d = nc.dram_tensor("x1b", [QROWS, C], BF16, kind="ExternalInput")
    x2b_d = nc.dram_tensor("x2b", [N2, C], BF16, kind="ExternalInput")
    wq_d = nc.dram_tensor("wq_t", [C, C], BF16, kind="ExternalInput")
    wk_d = nc.dram_tensor("wk_t", [C, C], BF16, kind="ExternalInput")
    wv_d = nc.dram_tensor("wv_t", [C, C], BF16, kind="ExternalInput")
    wo_d = nc.dram_tensor("wo_t", [C, C], BF16, kind="ExternalInput")
    wqo_d = nc.dram_tensor("wqo_t", [C, C], F32, kind="ExternalInput")
    bq_d = nc.dram_tensor("bq", [C], F32, kind="ExternalInput")
    bk_d = nc.dram_tensor("bk", [C], F32, kind="ExternalInput")
    bv_d = nc.dram_tensor("bv", [C], F32, kind="ExternalInput")
    bo2_d = nc.dram_tensor("bo2", [C], F32, kind="ExternalInput")
    out_d = nc.dram_tensor("out", [QROWS, C], F32, kind="ExternalOutput")

    with tile.TileContext(nc) as tc:
        with tc.tile_pool(name="cst", bufs=1) as cst, \
             tc.tile_pool(name="per", bufs=1) as per, \
             tc.tile_pool(name="sb", bufs=1) as sb, \
             tc.tile_pool(name="ps", bufs=1, space="PSUM") as ps:

            # ---- constants / weights ----
            ident = cst.tile([128, 128], F32)
            masks.make_identity(nc, ident[:])
            ones_bf = cst.tile([128, 128], BF16)
            nc.gpsimd.memset(ones_bf[:], 1.0)

            def load_w_bf(dram, nm):
                ts = []
                for cc in range(CCH):
                    t = cst.tile([128, C], BF16, name=f"{nm}{cc}", tag=f"{nm}{cc}")
                    nc.sync.dma_start(out=t[:], in_=dram[cc * 128:(cc + 1) * 128, :])
                    ts.append(t)
                return ts

            wk_t = load_w_bf(wk_d, "wk")
            wv_t = load_w_bf(wv_d, "wv")

            bk_t = []
            for d in range(CCH):
                t2 = cst.tile([128, 1], F32, name=f"bk{d}", tag=f"bk{d}")
                nc.sync.dma_start(out=t2[:], in_=bk_d[d * 128:(d + 1) * 128].unsqueeze(1))
                bk_t.append(t2)
            bv_bc = cst.tile([128, C], F32)
            nc.sync.dma_start(out=bv_bc[:], in_=bv_d[:].unsqueeze(0).broadcast_to([128, C]))

            # ---- persistent tensors ----
            kt_b = [per.tile([128, N2], BF16, name=f"ktb{cc}", tag=f"ktb{cc}")
                    for cc in range(CCH)]
            v_b = [per.tile([128, C], BF16, name=f"vb{i}", tag=f"vb{i}")
                   for i in range(KT)]

            # ---- phase X2: K^T and V ----
            for kc0 in range(N2 // 512):
                x2bt = []
                for cc in range(CCH):
                    t = sb.tile([128, 512], BF16, name=f"x2bt{cc}", tag=f"x2bt{cc}", bufs=3)
                    nc.sync.dma_start_transpose(
                        t[:], x2b_d[kc0 * 512:(kc0 + 1) * 512, cc * 128:(cc + 1) * 128])
                    x2bt.append(t)
                # K^T[d, k-block]
                for d in range(CCH):
                    pp = ps.tile([128, 512], F32, name="kps", tag="pB", bufs=3)
                    for cc in range(CCH):
                        nc.tensor.matmul(pp[:], lhsT=wk_t[cc][:, d * 128:(d + 1) * 128],
                                         rhs=x2bt[cc][:],
                                         start=(cc == 0), stop=(cc == CCH - 1))
                    nc.vector.tensor_add(
                        out=kt_b[d][:, kc0 * 512:(kc0 + 1) * 512],
                        in0=pp[:], in1=bk_t[d][:].broadcast_to([128, 512]))
                # V[k-subtile, :]
                for kb in range(4):
                    pp = ps.tile([128, C], F32, name="vps", tag="pB", bufs=3)
                    for cc in range(CCH):
                        nc.tensor.matmul(pp[:], lhsT=x2bt[cc][:, kb * 128:(kb + 1) * 128],
                                         rhs=wv_t[cc][:],
                                         start=(cc == 0), stop=(cc == CCH - 1))
                    nc.vector.tensor_add(out=v_b[kc0 * 4 + kb][:], in0=pp[:], in1=bv_bc[:])

            # ---- late weights: Q/Wqo/Wo paths (needed from chunk 0 on) ----
            wq_b = load_w_bf(wq_d, "wq")
            wqo_r = []
            for cc in range(CCH):
                stage2 = sb.tile([128, C], F32, name=f"wqos{cc}", tag="x1f1", bufs=2)
                nc.sync.dma_start(out=stage2[:], in_=wqo_d[cc * 128:(cc + 1) * 128, :])
                t2 = cst.tile([128, C], F32R, name=f"wqo{cc}", tag=f"wqo{cc}")
                nc.scalar.copy(t2[:], stage2[:])
                wqo_r.append(t2)
            wo_t = load_w_bf(wo_d, "wo")
            bq_t = []
            for d in range(CCH):
                t1 = cst.tile([128, 1], F32, name=f"bq{d}", tag=f"bq{d}")
                nc.sync.dma_start(out=t1[:], in_=bq_d[d * 128:(d + 1) * 128].unsqueeze(1))
                bq_t.append(t1)
            bo2_bc = cst.tile([128, C], F32)
            nc.sync.dma_start(out=bo2_bc[:], in_=bo2_d[:].unsqueeze(0).broadcast_to([128, C]))

            # ---- per q-chunk: transpose x1, Q^T, S^T/exp, rowsum, A^T, O ----
            for qc in range(NQC):
                q0 = qc * QC
                # x1 fp32 rows in, PE-transpose to x1t (f32r)
                x1f_in = []
                for rb in range(QC // 128):
                    t = sb.tile([128, C], F32, name=f"x1f{rb}", tag=f"x1f{rb}", bufs=2)
                    nc.sync.dma_start(out=t[:], in_=x1f_d[q0 + rb * 128:q0 + (rb + 1) * 128, :])
                    x1f_in.append(t)
                x1t_r = [sb.tile([128, QC], F32R, name=f"x1t{cc}", tag=f"x1t{cc}", bufs=2)
                         for cc in range(CCH)]
                for rb in range(QC // 128):
                    for cc in range(CCH):
                        tp = ps.tile([128, 128], F32, name="tps", tag="pA", bufs=3)
                        nc.tensor.transpose(tp[:], x1f_in[rb][:, cc * 128:(cc + 1) * 128],
                                            ident[:])
                        nc.scalar.copy(x1t_r[cc][:, rb * 128:(rb + 1) * 128], tp[:])
                # x1^T bf16 via xbar DMA for the Q projection
                x1bt = []
                for cc in range(CCH):
                    t = sb.tile([128, QC], BF16, name=f"x1bt{cc}", tag=f"x1bt{cc}", bufs=2)
                    nc.sync.dma_start_transpose(
                        t[:], x1b_d[q0:q0 + QC, cc * 128:(cc + 1) * 128])
                    x1bt.append(t)
                # Q^T (bf16) [d, q-chunk]
                qt_bf = []
                for d in range(CCH):
                    pp = ps.tile([128, QC], F32, name="qps", tag="pB", bufs=3)
                    for cc in range(CCH):
                        nc.tensor.matmul(pp[:], lhsT=wq_b[cc][:, d * 128:(d + 1) * 128],
                                         rhs=x1bt[cc][:],
                                         start=(cc == 0), stop=(cc == CCH - 1))
                    t = sb.tile([128, QC], BF16, name=f"qt{d}", tag=f"qt{d}", bufs=2)
                    nc.vector.tensor_add(out=t[:], in0=pp[:],
                                         in1=bq_t[d][:].broadcast_to([128, QC]))
                    qt_bf.append(t)
                # S^T tiles + exp -> pt[kt]
                pt = []
                for kt in range(KT):
                    pp = ps.tile([128, QC], F32, name="sps", tag="pA", bufs=3)
                    for cc in range(CCH):
                        nc.tensor.matmul(pp[:], lhsT=kt_b[cc][:, kt * 128:(kt + 1) * 128],
                                         rhs=qt_bf[cc][:],
                                         start=(cc == 0), stop=(cc == CCH - 1))
                    t = sb.tile([128, QC], BF16, name=f"pt{kt}", tag=f"pt{kt}", bufs=1)
                    nc.scalar.activation(t[:], pp[:], AF.Exp, scale=float(SCALE))
                    pt.append(t)
                # rowsum via ones-matmul over partitions, then reciprocal
                rs = ps.tile([128, QC], F32, name="rs", tag="pR", bufs=2)
                for kt in range(KT):
                    nc.tensor.matmul(rs[:], lhsT=ones_bf[:], rhs=pt[kt][:],
                                     start=(kt == 0), stop=(kt == KT - 1))
                recip = sb.tile([128, QC], F32, name="recip", tag="recip", bufs=2)
                nc.vector.reciprocal(recip[:], rs[:])
                # A^T [d, q-chunk]
                at_bf = []
                for d in range(CCH):
                    pp = ps.tile([128, QC], F32, name="aps", tag="pB", bufs=3)
                    for kt in range(KT):
                        nc.tensor.matmul(pp[:], lhsT=v_b[kt][:, d * 128:(d + 1) * 128],
                                         rhs=pt[kt][:],
                                         start=(kt == 0), stop=(kt == KT - 1))
                    t = sb.tile([128, QC], BF16, name=f"at{d}", tag=f"at{d}", bufs=2)
                    nc.vector.tensor_mul(out=t[:], in0=pp[:], in1=recip[:])
                    at_bf.append(t)
                # O = x1 @ Wqo^T (f32r) + A @ Wo^T (bf16) + bo2
                for rb in range(QC // 128):
                    pp = ps.tile([128, C], F32, name="ops", tag="pB", bufs=3)
                    for cc in range(CCH):
                        nc.tensor.matmul(pp[:], lhsT=x1t_r[cc][:, rb * 128:(rb + 1) * 128],
                                         rhs=wqo_r[cc][:],
                                         start=(cc == 0), stop=False)
                    for d in range(CCH):
                        nc.tensor.matmul(pp[:], lhsT=at_bf[d][:, rb * 128:(rb + 1) * 128],
                                         rhs=wo_t[d][:],
                                         start=False, stop=(d == CCH - 1))
                    ot = sb.tile([128, C], F32, name="ot", tag="ot", bufs=3)
                    nc.vector.tensor_add(out=ot[:], in0=pp[:], in1=bo2_bc[:])
                    nc.sync.dma_start(out=out_d[q0 + rb * 128:q0 + (rb + 1) * 128, :],
                                      in_=ot[:])

    nc.compile()
    return nc


def get_built():
    global _BUILT
    if _BUILT is None:
        _BUILT = build()
    return _BUILT


def make_in_maps(x1, x2, Wq, bq, Wk, bk, Wv, bv, Wo, bo):
    bf = ml_dtypes.bfloat16
    wq_t = np.ascontiguousarray(Wq.T).astype(bf)
    wk_t = np.ascontiguousarray(Wk.T).astype(bf)
    wv_t = np.ascontiguousarray(Wv.T).astype(bf)
    wo_t = np.ascontiguousarray(Wo.T).astype(bf)
    wqo_t = np.ascontiguousarray((Wo @ Wq).T).astype(np.float32)
    bo2 = (Wo @ bq + bo).astype(np.float32)
    in_maps = []
    for cid in range(NCORES):
        b, h = cid // 2, cid % 2
        x1s = np.ascontiguousarray(x1[b, h * QROWS:(h + 1) * QROWS, :])
        in_maps.append({
            "x1f": x1s,
            "x1b": x1s.astype(bf),
            "x2b": np.ascontiguousarray(x2[b]).astype(bf),
            "wq_t": wq_t, "wk_t": wk_t, "wv_t": wv_t, "wo_t": wo_t,
            "wqo_t": wqo_t,
            "bq": bq.astype(np.float32), "bk": bk.astype(np.float32),
            "bv": bv.astype(np.float32), "bo2": bo2,
        })
    return in_maps


LAST_RESULT = None


def kernel(x1, x2, Wq, bq, Wk, bk, Wv, bv, Wo, bo):
    global LAST_RESULT
    nc = get_built()
    in_maps = make_in_maps(x1, x2, Wq, bq, Wk, bk, Wv, bv, Wo, bo)
    trace = bool(os.environ.get("KERNEL_TRACE"))
    res = run_bass_kernel_spmd(nc, in_maps, core_ids=list(range(NCORES)), trace=trace)
    LAST_RESULT = res
    out = np.empty((B, N1, C), dtype=np.float32)
    for cid in range(NCORES):
        b, h = cid // 2, cid % 2
        out[b, h * QROWS:(h + 1) * QROWS, :] = res.results[cid]["out"]
    return out



# revision 2
# speedup vs baseline: 1.6865x; 1.6865x over previous
"""CrossFeatureAttention TRN2 kernel (fp8 DoubleRow attention).

Full inputs -> full output. Sharding: data-parallel over (batch b, half of N1)
across 8 cores; each core computes out[b, h*2048:(h+1)*2048, :].

Math (per core, x1 slice q=2048 rows, x2[b] k=4096 rows, C=512):
    Q  = x1 @ Wq^T + bq                      (bf16 matmul, kept in fp32)
    K  = x2 @ Wk^T + bk                      (fp8 DoubleRow)
    V  = x2 @ Wv^T + bv                      (fp8 DoubleRow)
    P  = exp(Q K^T / sqrt(C))                (S^T via fp8 DR; exp -> fp8)
    rs = colsum(P^T)  (ones fp8 DR matmul)
    A^T = V^T P^T / rs                       (fp8 DR)
    out = (Q + A) @ Wo^T + bo                (bf16, residual folded via qt reuse)

All attention-path operands are fp8e4 packed in DoubleRow pair layout
[128, 2, N] (two 128-deep contraction planes per matmul -> 2x PE rate).
Host supplies x1^T (bf16) and x2^T (fp8) so no on-device transposes.
"""

import os
import sys

import numpy as np

for _p in ("/root/.axon_site", "/root/.axon_site/_ro/trn_rl_repo",
           "/root/.axon_site/_ro/pypackages"):
    if _p not in sys.path and os.path.isdir(_p):
        sys.path.append(_p)

import ml_dtypes

import concourse.bacc as bacc
import concourse.mybir as mybir
import concourse.tile as tile
from concourse.bass_utils import run_bass_kernel_spmd

F32 = mybir.dt.float32
BF16 = mybir.dt.bfloat16
F8 = mybir.dt.float8e4
AF = mybir.ActivationFunctionType
DR = mybir.MatmulPerfMode.DoubleRow

B, N1, N2, C = 4, 4096, 4096, 512
NCORES = 8
QROWS = N1 * B // NCORES          # 2048 q rows per core
QC = 512                          # q-chunk (columns of S^T tiles)
NQC = QROWS // QC                 # 4 chunks
KT = N2 // 128                    # 32 k-tiles
CCH = C // 128                    # 4 contraction planes of 128
NKP = KT // 2                     # 16 k-plane pairs
SCALE = 1.0 / float(np.sqrt(C))

_BUILT = None


def build():
    nc = bacc.Bacc(None, target_bir_lowering=False, debug=False)

    x1t_d = nc.dram_tensor("x1t", [C, QROWS], BF16, kind="ExternalInput")
    x2t_d = nc.dram_tensor("x2t", [C, N2], F8, kind="ExternalInput")
    wq_d = nc.dram_tensor("wq_t", [C, C], BF16, kind="ExternalInput")
    wk_d = nc.dram_tensor("wk_t", [C, C], F8, kind="ExternalInput")
    wv_d = nc.dram_tensor("wv_t", [C, C], F8, kind="ExternalInput")
    wo_d = nc.dram_tensor("wo_t", [C, C], BF16, kind="ExternalInput")
    bq_d = nc.dram_tensor("bq", [C], F32, kind="ExternalInput")
    bk_d = nc.dram_tensor("bk", [C], F32, kind="ExternalInput")
    bv_d = nc.dram_tensor("bv", [C], F32, kind="ExternalInput")
    bo_d = nc.dram_tensor("bo", [C], F32, kind="ExternalInput")
    out_d = nc.dram_tensor("out", [QROWS, C], F32, kind="ExternalOutput")

    with tile.TileContext(nc) as tc:
        with tc.tile_pool(name="cst", bufs=1) as cst, \
             tc.tile_pool(name="per", bufs=1) as per, \
             tc.tile_pool(name="sb", bufs=1) as sb, \
             tc.tile_pool(name="ps", bufs=1, space="PSUM") as ps:

            # ---- constants / weights ----
            ones_f8 = cst.tile([128, 2, 128], F8)
            nc.gpsimd.memset(ones_f8[:], 1.0)

            def load_pair_f8(dram, nm):
                ts = []
                for j in range(2):
                    t = cst.tile([128, 2, C], F8, name=f"{nm}{j}", tag=f"{nm}{j}")
                    for i in range(2):
                        nc.sync.dma_start(
                            out=t[:, i, :],
                            in_=dram[(2 * j + i) * 128:(2 * j + i + 1) * 128, :])
                    ts.append(t)
                return ts

            wk_pair = load_pair_f8(wk_d, "wk")
            wv_pair = load_pair_f8(wv_d, "wv")

            bk_t = []
            for d in range(CCH):
                t2 = cst.tile([128, 1], F32, name=f"bk{d}", tag=f"bk{d}")
                nc.sync.dma_start(out=t2[:], in_=bk_d[d * 128:(d + 1) * 128].unsqueeze(1))
                bk_t.append(t2)
            bv_bc = cst.tile([128, C], F32)
            nc.sync.dma_start(out=bv_bc[:], in_=bv_d[:].unsqueeze(0).broadcast_to([128, C]))

            # ---- persistent tensors ----
            x2t_pair = [per.tile([128, 2, N2], F8, name=f"x2t{j}", tag=f"x2t{j}")
                        for j in range(2)]
            kt_pair = [per.tile([128, 2, N2], F8, name=f"ktp{j}", tag=f"ktp{j}")
                       for j in range(2)]
            v_pair = [per.tile([128, 2, C], F8, name=f"vp{j}", tag=f"vp{j}")
                      for j in range(NKP)]

            # ---- phase KV: K^T and V from x2^T (all fp8 DoubleRow) ----
            for kc0 in range(N2 // 512):
                ksl = slice(kc0 * 512, (kc0 + 1) * 512)
                for j in range(2):
                    for i in range(2):
                        nc.sync.dma_start(
                            out=x2t_pair[j][:, i, ksl],
                            in_=x2t_d[(2 * j + i) * 128:(2 * j + i + 1) * 128, ksl])
                # K^T[dd-plane, k-block]
                for dd in range(CCH):
                    pp = ps.tile([128, 512], F32, name="kps", tag="pB", bufs=3)
                    for j in range(2):
                        nc.tensor.matmul(pp[:],
                                         lhsT=wk_pair[j][:, :, dd * 128:(dd + 1) * 128],
                                         rhs=x2t_pair[j][:, :, ksl],
                                         start=(j == 0), stop=(j == 1),
                                         perf_mode=DR)
                    nc.vector.tensor_add(
                        out=kt_pair[dd // 2][:, dd % 2, ksl],
                        in0=pp[:], in1=bk_t[dd][:].broadcast_to([128, 512]))
                # V[k-subtile, :]
                for kb in range(4):
                    ki = kc0 * 4 + kb
                    pp = ps.tile([128, C], F32, name="vps", tag="pB", bufs=3)
                    for j in range(2):
                        nc.tensor.matmul(pp[:],
                                         lhsT=x2t_pair[j][:, :, ki * 128:(ki + 1) * 128],
                                         rhs=wv_pair[j][:],
                                         start=(j == 0), stop=(j == 1),
                                         perf_mode=DR)
                    nc.vector.tensor_add(out=v_pair[ki // 2][:, ki % 2, :],
                                         in0=pp[:], in1=bv_bc[:])

            # ---- late weights: Q/Wo paths (needed from chunk 0 on) ----
            wq_b = []
            for cc in range(CCH):
                t = cst.tile([128, C], BF16, name=f"wq{cc}", tag=f"wq{cc}")
                nc.sync.dma_start(out=t[:], in_=wq_d[cc * 128:(cc + 1) * 128, :])
                wq_b.append(t)
            wo_b = []
            for cc in range(CCH):
                t = cst.tile([128, C], BF16, name=f"wo{cc}", tag=f"wo{cc}")
                nc.sync.dma_start(out=t[:], in_=wo_d[cc * 128:(cc + 1) * 128, :])
                wo_b.append(t)
            bq_t = []
            for d in range(CCH):
                t1 = cst.tile([128, 1], F32, name=f"bq{d}", tag=f"bq{d}")
                nc.sync.dma_start(out=t1[:], in_=bq_d[d * 128:(d + 1) * 128].unsqueeze(1))
                bq_t.append(t1)
            bo_bc = cst.tile([128, C], F32)
            nc.sync.dma_start(out=bo_bc[:], in_=bo_d[:].unsqueeze(0).broadcast_to([128, C]))

            # ---- per q-chunk: Q^T, S^T/exp, rowsum, A^T, O ----
            for qc in range(NQC):
                q0 = qc * QC
                # x1^T bf16 slices for the Q projection
                x1bt = []
                for cc in range(CCH):
                    t = sb.tile([128, QC], BF16, name=f"x1bt{cc}", tag=f"x1bt{cc}", bufs=2)
                    nc.sync.dma_start(out=t[:],
                                      in_=x1t_d[cc * 128:(cc + 1) * 128, q0:q0 + QC])
                    x1bt.append(t)
                # Q^T [dd, q-chunk]: fp32 copy for residual + fp8 copy for scores
                qt_f32 = []
                qt_f8 = [sb.tile([128, 2, QC], F8, name=f"qt8_{j}", tag=f"qt8_{j}", bufs=2)
                         for j in range(2)]
                for dd in range(CCH):
                    pp = ps.tile([128, QC], F32, name="qps", tag="pB", bufs=3)
                    for cc in range(CCH):
                        nc.tensor.matmul(pp[:], lhsT=wq_b[cc][:, dd * 128:(dd + 1) * 128],
                                         rhs=x1bt[cc][:],
                                         start=(cc == 0), stop=(cc == CCH - 1))
                    t = sb.tile([128, QC], F32, name=f"qtf{dd}", tag=f"qtf{dd}", bufs=2)
                    nc.vector.tensor_add(out=t[:], in0=pp[:],
                                         in1=bq_t[dd][:].broadcast_to([128, QC]))
                    qt_f32.append(t)
                    nc.scalar.copy(qt_f8[dd // 2][:, dd % 2, :], t[:])
                # S^T tiles + exp -> pt pairs (fp8)
                pt_pair = [sb.tile([128, 2, QC], F8, name=f"pt{j}", tag=f"pt{j}", bufs=1)
                           for j in range(NKP)]
                for kt in range(KT):
                    pp = ps.tile([128, QC], F32, name="sps", tag="pA", bufs=3)
                    for j in range(2):
                        nc.tensor.matmul(pp[:],
                                         lhsT=kt_pair[j][:, :, kt * 128:(kt + 1) * 128],
                                         rhs=qt_f8[j][:],
                                         start=(j == 0), stop=(j == 1),
                                         perf_mode=DR)
                    nc.scalar.activation(pt_pair[kt // 2][:, kt % 2, :], pp[:],
                                         AF.Exp, scale=float(SCALE))
                # rowsum via ones-matmul over partitions, then reciprocal
                rs = ps.tile([128, QC], F32, name="rs", tag="pR", bufs=2)
                for j in range(NKP):
                    nc.tensor.matmul(rs[:], lhsT=ones_f8[:], rhs=pt_pair[j][:],
                                     start=(j == 0), stop=(j == NKP - 1),
                                     perf_mode=DR)
                recip = sb.tile([128, QC], F32, name="recip", tag="recip", bufs=2)
                nc.vector.reciprocal(recip[:], rs[:])
                # A^T [dd, q-chunk] + residual fold -> qa (bf16)
                qa_bf = []
                for dd in range(CCH):
                    pp = ps.tile([128, QC], F32, name="aps", tag="pB", bufs=3)
                    for j in range(NKP):
                        nc.tensor.matmul(pp[:],
                                         lhsT=v_pair[j][:, :, dd * 128:(dd + 1) * 128],
                                         rhs=pt_pair[j][:],
                                         start=(j == 0), stop=(j == NKP - 1),
                                         perf_mode=DR)
                    at = sb.tile([128, QC], F32, name="at", tag="at", bufs=2)
                    nc.vector.tensor_mul(out=at[:], in0=pp[:], in1=recip[:])
                    t = sb.tile([128, QC], BF16, name=f"qa{dd}", tag=f"qa{dd}", bufs=2)
                    nc.vector.tensor_add(out=t[:], in0=at[:], in1=qt_f32[dd][:])
                    qa_bf.append(t)
                # O = (Q + A) @ Wo^T + bo
                for rb in range(QC // 128):
                    pp = ps.tile([128, C], F32, name="ops", tag="pB", bufs=3)
                    for cc in range(CCH):
                        nc.tensor.matmul(pp[:], lhsT=qa_bf[cc][:, rb * 128:(rb + 1) * 128],
                                         rhs=wo_b[cc][:],
                                         start=(cc == 0), stop=(cc == CCH - 1))
                    ot = sb.tile([128, C], F32, name="ot", tag="ot", bufs=3)
                    nc.vector.tensor_add(out=ot[:], in0=pp[:], in1=bo_bc[:])
                    nc.sync.dma_start(out=out_d[q0 + rb * 128:q0 + (rb + 1) * 128, :],
                                      in_=ot[:])

    nc.compile()
    return nc


def get_built():
    global _BUILT
    if _BUILT is None:
        _BUILT = build()
    return _BUILT


def make_in_maps(x1, x2, Wq, bq, Wk, bk, Wv, bv, Wo, bo):
    bf = ml_dtypes.bfloat16
    f8 = ml_dtypes.float8_e4m3
    wq_t = np.ascontiguousarray(Wq.T).astype(bf)
    wk_t = np.ascontiguousarray(Wk.T).astype(f8)
    wv_t = np.ascontiguousarray(Wv.T).astype(f8)
    wo_t = np.ascontiguousarray(Wo.T).astype(bf)
    x2t_b = [np.ascontiguousarray(x2[b].T).astype(f8) for b in range(B)]
    in_maps = []
    for cid in range(NCORES):
        b, h = cid // 2, cid % 2
        x1t = np.ascontiguousarray(x1[b, h * QROWS:(h + 1) * QROWS, :].T).astype(bf)
        in_maps.append({
            "x1t": x1t,
            "x2t": x2t_b[b],
            "wq_t": wq_t, "wk_t": wk_t, "wv_t": wv_t, "wo_t": wo_t,
            "bq": bq.astype(np.float32), "bk": bk.astype(np.float32),
            "bv": bv.astype(np.float32), "bo": bo.astype(np.float32),
        })
    return in_maps


LAST_RESULT = None


def kernel(x1, x2, Wq, bq, Wk, bk, Wv, bv, Wo, bo):
    global LAST_RESULT
    nc = get_built()
    in_maps = make_in_maps(x1, x2, Wq, bq, Wk, bk, Wv, bv, Wo, bo)
    trace = bool(os.environ.get("KERNEL_TRACE"))
    res = run_bass_kernel_spmd(nc, in_maps, core_ids=list(range(NCORES)), trace=trace)
    LAST_RESULT = res
    out = np.empty((B, N1, C), dtype=np.float32)
    for cid in range(NCORES):
        b, h = cid // 2, cid % 2
        out[b, h * QROWS:(h + 1) * QROWS, :] = res.results[cid]["out"]
    return out


# revision 3
# speedup vs baseline: 1.8812x; 1.1154x over previous
"""CrossFeatureAttention TRN2 kernel (fp8 DoubleRow attention).

Full inputs -> full output. Sharding: data-parallel over (batch b, half of N1)
across 8 cores; each core computes out[b, h*2048:(h+1)*2048, :].

Math (per core, x1 slice q=2048 rows, x2[b] k=4096 rows, C=512):
    Q  = x1 @ Wq^T + bq                      (bf16 matmul, kept in fp32)
    K  = x2 @ Wk^T          (bk dropped: per-q constant in scores -> softmax inv)
    V  = x2 @ Wv^T          (bv folded into bo2 = Wo bv + bo: softmax rows sum 1)
    P  = exp(Q K^T / sqrt(C))                (S^T via fp8 DR; exp -> fp8)
    rs = colsum(P^T)  (ones fp8 DR matmul)
    A^T = V^T P^T / rs                       (fp8 DR)
    out = (Q + A) @ Wo^T + bo2               (bf16, residual folded via qt reuse)

All attention-path operands are fp8e4 packed in DoubleRow pair layout
[128, 2, N] (two 128-deep contraction planes per matmul -> 2x PE rate).
Host supplies x1^T (bf16) and x2^T (fp8) so no on-device transposes.
Engine balance: PE matmuls; scalar does exp + K-copies + qt fp8; vector does
V-copies, Q bias, A normalize, O bias; psum->fp8 copies overlap KV matmuls.
"""

import os
import sys

import numpy as np

for _p in ("/root/.axon_site", "/root/.axon_site/_ro/trn_rl_repo",
           "/root/.axon_site/_ro/pypackages"):
    if _p not in sys.path and os.path.isdir(_p):
        sys.path.append(_p)

import ml_dtypes

import concourse.bacc as bacc
import concourse.mybir as mybir
import concourse.tile as tile
from concourse.bass_utils import run_bass_kernel_spmd

F32 = mybir.dt.float32
BF16 = mybir.dt.bfloat16
F8 = mybir.dt.float8e4
AF = mybir.ActivationFunctionType
DR = mybir.MatmulPerfMode.DoubleRow

B, N1, N2, C = 4, 4096, 4096, 512
NCORES = 8
QROWS = N1 * B // NCORES          # 2048 q rows per core
QC = 512                          # q-chunk (columns of S^T tiles)
NQC = QROWS // QC                 # 4 chunks
KT = N2 // 128                    # 32 k-tiles
CCH = C // 128                    # 4 contraction planes of 128
NKP = KT // 2                     # 16 k-plane pairs
SCALE = 1.0 / float(np.sqrt(C))

_BUILT = None


def build():
    nc = bacc.Bacc(None, target_bir_lowering=False, debug=False)

    x1t_d = nc.dram_tensor("x1t", [C, QROWS], BF16, kind="ExternalInput")
    x2t_d = nc.dram_tensor("x2t", [C, N2], F8, kind="ExternalInput")
    wq_d = nc.dram_tensor("wq_t", [C, C], BF16, kind="ExternalInput")
    wk_d = nc.dram_tensor("wk_t", [C, C], F8, kind="ExternalInput")
    wv_d = nc.dram_tensor("wv_t", [C, C], F8, kind="ExternalInput")
    wo_d = nc.dram_tensor("wo_t", [C, C], BF16, kind="ExternalInput")
    bq_d = nc.dram_tensor("bq", [C], F32, kind="ExternalInput")
    bo2_d = nc.dram_tensor("bo2", [C], F32, kind="ExternalInput")
    out_d = nc.dram_tensor("out", [QROWS, C], F32, kind="ExternalOutput")

    with tile.TileContext(nc) as tc:
        with tc.tile_pool(name="cst", bufs=1) as cst, \
             tc.tile_pool(name="per", bufs=1) as per, \
             tc.tile_pool(name="sb", bufs=1) as sb, \
             tc.tile_pool(name="ps", bufs=1, space="PSUM") as ps:

            # ---- constants / weights ----
            ones_f8 = cst.tile([128, 2, 128], F8)
            nc.gpsimd.memset(ones_f8[:], 1.0)

            def load_pair_f8(dram, nm):
                ts = []
                for j in range(2):
                    t = cst.tile([128, 2, C], F8, name=f"{nm}{j}", tag=f"{nm}{j}")
                    for i in range(2):
                        nc.sync.dma_start(
                            out=t[:, i, :],
                            in_=dram[(2 * j + i) * 128:(2 * j + i + 1) * 128, :])
                    ts.append(t)
                return ts

            wk_pair = load_pair_f8(wk_d, "wk")
            wv_pair = load_pair_f8(wv_d, "wv")

            # ---- persistent tensors ----
            x2t_pair = [per.tile([128, 2, N2], F8, name=f"x2t{j}", tag=f"x2t{j}")
                        for j in range(2)]
            kt_pair = [per.tile([128, 2, N2], F8, name=f"ktp{j}", tag=f"ktp{j}")
                       for j in range(2)]
            v_pair = [per.tile([128, 2, C], F8, name=f"vp{j}", tag=f"vp{j}")
                      for j in range(NKP)]

            # ---- phase KV: K^T and V from x2^T (all fp8 DoubleRow) ----
            for kc0 in range(N2 // 512):
                ksl = slice(kc0 * 512, (kc0 + 1) * 512)
                for j in range(2):
                    for i in range(2):
                        nc.sync.dma_start(
                            out=x2t_pair[j][:, i, ksl],
                            in_=x2t_d[(2 * j + i) * 128:(2 * j + i + 1) * 128, ksl])
                # K^T[dd-plane, k-block]; bk dropped (softmax-invariant)
                for dd in range(CCH):
                    pp = ps.tile([128, 512], F32, name="kps", tag="pB", bufs=3)
                    for j in range(2):
                        nc.tensor.matmul(pp[:],
                                         lhsT=wk_pair[j][:, :, dd * 128:(dd + 1) * 128],
                                         rhs=x2t_pair[j][:, :, ksl],
                                         start=(j == 0), stop=(j == 1),
                                         perf_mode=DR)
                    nc.scalar.copy(kt_pair[dd // 2][:, dd % 2, ksl], pp[:])
                # V[k-subtile, :]; bv folded into bo2
                for kb in range(4):
                    ki = kc0 * 4 + kb
                    pp = ps.tile([128, C], F32, name="vps", tag="pB", bufs=3)
                    for j in range(2):
                        nc.tensor.matmul(pp[:],
                                         lhsT=x2t_pair[j][:, :, ki * 128:(ki + 1) * 128],
                                         rhs=wv_pair[j][:],
                                         start=(j == 0), stop=(j == 1),
                                         perf_mode=DR)
                    nc.vector.tensor_copy(out=v_pair[ki // 2][:, ki % 2, :], in_=pp[:])

            # ---- late weights: Q/Wo paths (needed from chunk 0 on) ----
            wq_b = []
            for cc in range(CCH):
                t = cst.tile([128, C], BF16, name=f"wq{cc}", tag=f"wq{cc}")
                nc.sync.dma_start(out=t[:], in_=wq_d[cc * 128:(cc + 1) * 128, :])
                wq_b.append(t)
            wo_b = []
            for cc in range(CCH):
                t = cst.tile([128, C], BF16, name=f"wo{cc}", tag=f"wo{cc}")
                nc.sync.dma_start(out=t[:], in_=wo_d[cc * 128:(cc + 1) * 128, :])
                wo_b.append(t)
            bq_t = []
            for d in range(CCH):
                t1 = cst.tile([128, 1], F32, name=f"bq{d}", tag=f"bq{d}")
                nc.sync.dma_start(out=t1[:], in_=bq_d[d * 128:(d + 1) * 128].unsqueeze(1))
                bq_t.append(t1)
            bo_bc = cst.tile([128, C], F32)
            nc.sync.dma_start(out=bo_bc[:], in_=bo2_d[:].unsqueeze(0).broadcast_to([128, C]))

            # ---- Q^T projection for one chunk (PE + vector f32 + scalar fp8) ----
            def emit_qproj(qc):
                q0 = qc * QC
                x1bt = []
                for cc in range(CCH):
                    t = sb.tile([128, QC], BF16, name=f"x1bt{cc}", tag=f"x1bt{cc}", bufs=2)
                    nc.sync.dma_start(out=t[:],
                                      in_=x1t_d[cc * 128:(cc + 1) * 128, q0:q0 + QC])
                    x1bt.append(t)
                qt_f32 = []
                qt_f8 = [sb.tile([128, 2, QC], F8, name=f"qt8_{j}", tag=f"qt8_{j}", bufs=2)
                         for j in range(2)]
                for dd in range(CCH):
                    pp = ps.tile([128, QC], F32, name="qps", tag="pB", bufs=3)
                    for cc in range(CCH):
                        nc.tensor.matmul(pp[:], lhsT=wq_b[cc][:, dd * 128:(dd + 1) * 128],
                                         rhs=x1bt[cc][:],
                                         start=(cc == 0), stop=(cc == CCH - 1))
                    t = sb.tile([128, QC], F32, name=f"qtf{dd}", tag=f"qtf{dd}", bufs=2)
                    nc.vector.tensor_add(out=t[:], in0=pp[:],
                                         in1=bq_t[dd][:].broadcast_to([128, QC]))
                    qt_f32.append(t)
                    nc.scalar.activation(qt_f8[dd // 2][:, dd % 2, :], pp[:],
                                         AF.Identity, bias=bq_t[dd][:])
                return qt_f32, qt_f8

            qt_cur = emit_qproj(0)

            # ---- per q-chunk: S^T/exp, rowsum, A^T, (next Q), O ----
            for qc in range(NQC):
                q0 = qc * QC
                qt_f32, qt_f8 = qt_cur
                # S^T tiles + exp -> pt pairs (fp8)
                pt_pair = [sb.tile([128, 2, QC], F8, name=f"pt{j}", tag=f"pt{j}", bufs=1)
                           for j in range(NKP)]
                for kt in range(KT):
                    pp = ps.tile([128, QC], F32, name="sps", tag="pA", bufs=3)
                    for j in range(2):
                        nc.tensor.matmul(pp[:],
                                         lhsT=kt_pair[j][:, :, kt * 128:(kt + 1) * 128],
                                         rhs=qt_f8[j][:],
                                         start=(j == 0), stop=(j == 1),
                                         perf_mode=DR)
                    nc.scalar.activation(pt_pair[kt // 2][:, kt % 2, :], pp[:],
                                         AF.Exp, scale=float(SCALE))
                # rowsum via ones-matmul over partitions, then reciprocal
                rs = ps.tile([128, QC], F32, name="rs", tag="pR", bufs=2)
                for j in range(NKP):
                    nc.tensor.matmul(rs[:], lhsT=ones_f8[:], rhs=pt_pair[j][:],
                                     start=(j == 0), stop=(j == NKP - 1),
                                     perf_mode=DR)
                recip = sb.tile([128, QC], F32, name="recip", tag="recip", bufs=2)
                rscr = sb.tile([128, QC], F32, name="rscr", tag="rscr", bufs=2)
                nc.vector.reciprocal_approx_accurate(out=recip[:], in_=rs[:],
                                                     scratch=rscr[:])
                # A^T [dd, q-chunk] + residual fold -> qa (bf16)
                qa_bf = []
                for dd in range(CCH):
                    pp = ps.tile([128, QC], F32, name="aps", tag="pB", bufs=3)
                    for j in range(NKP):
                        nc.tensor.matmul(pp[:],
                                         lhsT=v_pair[j][:, :, dd * 128:(dd + 1) * 128],
                                         rhs=pt_pair[j][:],
                                         start=(j == 0), stop=(j == NKP - 1),
                                         perf_mode=DR)
                    at = sb.tile([128, QC], F32, name="at", tag="at", bufs=2)
                    nc.vector.tensor_mul(out=at[:], in0=pp[:], in1=recip[:])
                    t = sb.tile([128, QC], BF16, name=f"qa{dd}", tag=f"qa{dd}", bufs=2)
                    nc.vector.tensor_add(out=t[:], in0=at[:], in1=qt_f32[dd][:])
                    qa_bf.append(t)
                # next chunk's Q projection fills the PE during the vector tail
                if qc + 1 < NQC:
                    qt_cur = emit_qproj(qc + 1)
                # O = (Q + A) @ Wo^T + bo2
                for rb in range(QC // 128):
                    pp = ps.tile([128, C], F32, name="ops", tag="pB", bufs=3)
                    for cc in range(CCH):
                        nc.tensor.matmul(pp[:], lhsT=qa_bf[cc][:, rb * 128:(rb + 1) * 128],
                                         rhs=wo_b[cc][:],
                                         start=(cc == 0), stop=(cc == CCH - 1))
                    ot = sb.tile([128, C], F32, name="ot", tag="ot", bufs=3)
                    nc.vector.tensor_add(out=ot[:], in0=pp[:], in1=bo_bc[:])
                    nc.sync.dma_start(out=out_d[q0 + rb * 128:q0 + (rb + 1) * 128, :],
                                      in_=ot[:])

    nc.compile()
    return nc


def get_built():
    global _BUILT
    if _BUILT is None:
        _BUILT = build()
    return _BUILT


def make_in_maps(x1, x2, Wq, bq, Wk, bk, Wv, bv, Wo, bo):
    bf = ml_dtypes.bfloat16
    f8 = ml_dtypes.float8_e4m3
    wq_t = np.ascontiguousarray(Wq.T).astype(bf)
    wk_t = np.ascontiguousarray(Wk.T).astype(f8)
    wv_t = np.ascontiguousarray(Wv.T).astype(f8)
    wo_t = np.ascontiguousarray(Wo.T).astype(bf)
    bo2 = (Wo @ bv + bo).astype(np.float32)
    x2t_b = [np.ascontiguousarray(x2[b].T).astype(f8) for b in range(B)]
    in_maps = []
    for cid in range(NCORES):
        b, h = cid // 2, cid % 2
        x1t = np.ascontiguousarray(x1[b, h * QROWS:(h + 1) * QROWS, :].T).astype(bf)
        in_maps.append({
            "x1t": x1t,
            "x2t": x2t_b[b],
            "wq_t": wq_t, "wk_t": wk_t, "wv_t": wv_t, "wo_t": wo_t,
            "bq": bq.astype(np.float32), "bo2": bo2,
        })
    return in_maps


LAST_RESULT = None


def kernel(x1, x2, Wq, bq, Wk, bk, Wv, bv, Wo, bo):
    global LAST_RESULT
    nc = get_built()
    in_maps = make_in_maps(x1, x2, Wq, bq, Wk, bk, Wv, bv, Wo, bo)
    trace = bool(os.environ.get("KERNEL_TRACE"))
    res = run_bass_kernel_spmd(nc, in_maps, core_ids=list(range(NCORES)), trace=trace)
    LAST_RESULT = res
    out = np.empty((B, N1, C), dtype=np.float32)
    for cid in range(NCORES):
        b, h = cid // 2, cid % 2
        out[b, h * QROWS:(h + 1) * QROWS, :] = res.results[cid]["out"]
    return out


# revision 5
# speedup vs baseline: 1.9764x; 1.0506x over previous
"""CrossFeatureAttention TRN2 kernel (fp8 DoubleRow attention, software-pipelined).

Full inputs -> full output. Sharding: data-parallel over (batch b, half of N1)
across 8 cores; each core computes out[b, h*2048:(h+1)*2048, :].

Math (per core, x1 slice q=2048 rows, x2[b] k=4096 rows, C=512):
    Q  = x1 @ Wq^T + bq                      (bf16 matmul, kept in fp32)
    K  = x2 @ Wk^T          (bk dropped: per-q constant in scores -> softmax inv)
    V  = x2 @ Wv^T          (bv folded into bo2 = Wo bv + bo: softmax rows sum 1)
    P  = exp(Q K^T / sqrt(C))                (S^T via fp8 DR; exp -> fp8)
    rs = colsum(P^T)  (ones fp8 DR matmul)
    A^T = V^T P^T / rs                       (fp8 DR)
    out = (Q + A) @ Wo^T + bo2               (bf16, residual folded via qt reuse)

All attention-path operands are fp8e4 packed in DoubleRow pair layout
[128, 2, N] (two 128-deep contraction planes per matmul -> 2x PE rate).
Host supplies x1^T (bf16) and x2^T (fp8) so no on-device transposes.

Pipelining: chunk 0's S/exp tiles are interleaved into the KV loop (each
k-block's K columns become S-ready immediately); chunk qc's A phase is fused
with chunk qc+1's S phase so the scalar exp stream always hides under PE
work. psum->fp8 copies are split across scalar and vector.
"""

import os
import sys

import numpy as np

for _p in ("/root/.axon_site", "/root/.axon_site/_ro/trn_rl_repo",
           "/root/.axon_site/_ro/pypackages"):
    if _p not in sys.path and os.path.isdir(_p):
        sys.path.append(_p)

import ml_dtypes

import concourse.bacc as bacc
import concourse.mybir as mybir
import concourse.tile as tile
from concourse.bass_utils import run_bass_kernel_spmd

F32 = mybir.dt.float32
BF16 = mybir.dt.bfloat16
F8 = mybir.dt.float8e4
AF = mybir.ActivationFunctionType
DR = mybir.MatmulPerfMode.DoubleRow

B, N1, N2, C = 4, 4096, 4096, 512
NCORES = 8
QROWS = N1 * B // NCORES          # 2048 q rows per core
QC = 512                          # q-chunk (columns of S^T tiles)
NQC = QROWS // QC                 # 4 chunks
KT = N2 // 128                    # 32 k-tiles
CCH = C // 128                    # 4 contraction planes of 128
NKP = KT // 2                     # 16 k-plane pairs
SCALE = 1.0 / float(np.sqrt(C))

_BUILT = None


def build():
    nc = bacc.Bacc(None, target_bir_lowering=False, debug=False)

    x1t_d = nc.dram_tensor("x1t", [C, QROWS], BF16, kind="ExternalInput")
    x2t_d = nc.dram_tensor("x2t", [C, N2], F8, kind="ExternalInput")
    wq_d = nc.dram_tensor("wq_t", [C, C], BF16, kind="ExternalInput")
    wk_d = nc.dram_tensor("wk_t", [C, C], F8, kind="ExternalInput")
    wv_d = nc.dram_tensor("wv_t", [C, C], F8, kind="ExternalInput")
    wo_d = nc.dram_tensor("wo_t", [C, C], BF16, kind="ExternalInput")
    bq_d = nc.dram_tensor("bq", [C], F32, kind="ExternalInput")
    bo2_d = nc.dram_tensor("bo2", [C], F32, kind="ExternalInput")
    out_d = nc.dram_tensor("out", [QROWS, C], F32, kind="ExternalOutput")

    with tile.TileContext(nc) as tc:
        with tc.tile_pool(name="cst", bufs=1) as cst, \
             tc.tile_pool(name="per", bufs=1) as per, \
             tc.tile_pool(name="sb", bufs=1) as sb, \
             tc.tile_pool(name="ps", bufs=1, space="PSUM") as ps:

            # ---- constants / weights ----
            ones_f8 = cst.tile([128, 2, 128], F8)
            nc.gpsimd.memset(ones_f8[:], 1.0)

            def load_pair_f8(dram, nm):
                ts = []
                for j in range(2):
                    t = cst.tile([128, 2, C], F8, name=f"{nm}{j}", tag=f"{nm}{j}")
                    for i in range(2):
                        nc.sync.dma_start(
                            out=t[:, i, :],
                            in_=dram[(2 * j + i) * 128:(2 * j + i + 1) * 128, :])
                    ts.append(t)
                return ts

            wk_pair = load_pair_f8(wk_d, "wk")
            wv_pair = load_pair_f8(wv_d, "wv")

            wq_b = []
            for cc in range(CCH):
                t = cst.tile([128, C], BF16, name=f"wq{cc}", tag=f"wq{cc}")
                nc.sync.dma_start(out=t[:], in_=wq_d[cc * 128:(cc + 1) * 128, :])
                wq_b.append(t)
            bq_t = []
            for d in range(CCH):
                t1 = cst.tile([128, 1], F32, name=f"bq{d}", tag=f"bq{d}")
                nc.sync.dma_start(out=t1[:], in_=bq_d[d * 128:(d + 1) * 128].unsqueeze(1))
                bq_t.append(t1)

            # ---- persistent tensors ----
            x2t_pair = [per.tile([128, 2, N2], F8, name=f"x2t{j}", tag=f"x2t{j}")
                        for j in range(2)]
            kt_pair = [per.tile([128, 2, N2], F8, name=f"ktp{j}", tag=f"ktp{j}")
                       for j in range(2)]
            v_pair = [per.tile([128, 2, C], F8, name=f"vp{j}", tag=f"vp{j}")
                      for j in range(NKP)]

            # ---- Q^T projection for one chunk (PE + vector f32 + scalar fp8) ----
            def emit_qproj(qc):
                q0 = qc * QC
                x1bt = []
                for cc in range(CCH):
                    t = sb.tile([128, QC], BF16, name=f"x1bt{cc}", tag=f"x1bt{cc}", bufs=2)
                    nc.sync.dma_start(out=t[:],
                                      in_=x1t_d[cc * 128:(cc + 1) * 128, q0:q0 + QC])
                    x1bt.append(t)
                qt_f32 = []
                qt_f8 = [sb.tile([128, 2, QC], F8, name=f"qt8_{j}", tag=f"qt8_{j}", bufs=2)
                         for j in range(2)]
                for dd in range(CCH):
                    pp = ps.tile([128, QC], F32, name="qps", tag="pB", bufs=3)
                    for cc in range(CCH):
                        nc.tensor.matmul(pp[:], lhsT=wq_b[cc][:, dd * 128:(dd + 1) * 128],
                                         rhs=x1bt[cc][:],
                                         start=(cc == 0), stop=(cc == CCH - 1))
                    t = sb.tile([128, QC], F32, name=f"qtf{dd}", tag=f"qtf{dd}", bufs=2)
                    nc.vector.tensor_add(out=t[:], in0=pp[:],
                                         in1=bq_t[dd][:].broadcast_to([128, QC]))
                    qt_f32.append(t)
                    nc.scalar.activation(qt_f8[dd // 2][:, dd % 2, :], pp[:],
                                         AF.Identity, bias=bq_t[dd][:])
                return qt_f32, qt_f8

            def alloc_pt():
                return [sb.tile([128, 2, QC], F8, name=f"pt{j}", tag=f"pt{j}", bufs=2)
                        for j in range(NKP)]

            def emit_s_tile(kt, qt_f8, pt_pair):
                pp = ps.tile([128, QC], F32, name="sps", tag="pA", bufs=3)
                for j in range(2):
                    nc.tensor.matmul(pp[:],
                                     lhsT=kt_pair[j][:, :, kt * 128:(kt + 1) * 128],
                                     rhs=qt_f8[j][:],
                                     start=(j == 0), stop=(j == 1),
                                     perf_mode=DR)
                nc.scalar.activation(pt_pair[kt // 2][:, kt % 2, :], pp[:],
                                     AF.Exp, scale=float(SCALE))

            qt_cur = emit_qproj(0)
            pt_cur = alloc_pt()

            # ---- phase KV (+ chunk-0 S/exp interleaved) ----
            for kc0 in range(N2 // 512):
                ksl = slice(kc0 * 512, (kc0 + 1) * 512)
                for j in range(2):
                    for i in range(2):
                        nc.sync.dma_start(
                            out=x2t_pair[j][:, i, ksl],
                            in_=x2t_d[(2 * j + i) * 128:(2 * j + i + 1) * 128, ksl])
                # K^T[dd-plane, k-block]; bk dropped (softmax-invariant)
                for dd in range(CCH):
                    pp = ps.tile([128, 512], F32, name="kps", tag="pB", bufs=3)
                    for j in range(2):
                        nc.tensor.matmul(pp[:],
                                         lhsT=wk_pair[j][:, :, dd * 128:(dd + 1) * 128],
                                         rhs=x2t_pair[j][:, :, ksl],
                                         start=(j == 0), stop=(j == 1),
                                         perf_mode=DR)
                    if dd < 2:
                        nc.scalar.copy(kt_pair[dd // 2][:, dd % 2, ksl], pp[:])
                    else:
                        nc.vector.tensor_copy(out=kt_pair[dd // 2][:, dd % 2, ksl],
                                              in_=pp[:])
                # V[k-subtile, :]; bv folded into bo2
                for kb in range(4):
                    ki = kc0 * 4 + kb
                    pp = ps.tile([128, C], F32, name="vps", tag="pB", bufs=3)
                    for j in range(2):
                        nc.tensor.matmul(pp[:],
                                         lhsT=x2t_pair[j][:, :, ki * 128:(ki + 1) * 128],
                                         rhs=wv_pair[j][:],
                                         start=(j == 0), stop=(j == 1),
                                         perf_mode=DR)
                    nc.vector.tensor_copy(out=v_pair[ki // 2][:, ki % 2, :], in_=pp[:])
                # chunk 0's S tiles for the k-blocks just produced
                for kt in range(kc0 * 4, kc0 * 4 + 4):
                    emit_s_tile(kt, qt_cur[1], pt_cur)

            # ---- late weights for the O path ----
            wo_b = []
            for cc in range(CCH):
                t = cst.tile([128, C], BF16, name=f"wo{cc}", tag=f"wo{cc}")
                nc.sync.dma_start(out=t[:], in_=wo_d[cc * 128:(cc + 1) * 128, :])
                wo_b.append(t)
            bo_bc = cst.tile([128, C], F32)
            nc.sync.dma_start(out=bo_bc[:], in_=bo2_d[:].unsqueeze(0).broadcast_to([128, C]))

            # ---- per q-chunk: (next Q), rowsum, A fused with next S, O ----
            for qc in range(NQC):
                q0 = qc * QC
                qt_f32, qt_f8 = qt_cur
                pt_pair = pt_cur
                if qc + 1 < NQC:
                    qt_cur = emit_qproj(qc + 1)
                    pt_cur = alloc_pt()
                # rowsum via ones-matmul over partitions, then reciprocal
                rs = ps.tile([128, QC], F32, name="rs", tag="pR", bufs=2)
                for j in range(NKP):
                    nc.tensor.matmul(rs[:], lhsT=ones_f8[:], rhs=pt_pair[j][:],
                                     start=(j == 0), stop=(j == NKP - 1),
                                     perf_mode=DR)
                recip = sb.tile([128, QC], F32, name="recip", tag="recip", bufs=2)
                rscr = sb.tile([128, QC], F32, name="rscr", tag="rscr", bufs=2)
                nc.vector.reciprocal_approx_accurate(out=recip[:], in_=rs[:],
                                                     scratch=rscr[:])
                # A^T groups fused with next chunk's S/exp stream
                qa_bf = []
                for dd in range(CCH):
                    pp = ps.tile([128, QC], F32, name="aps", tag="pB", bufs=3)
                    for j in range(NKP):
                        nc.tensor.matmul(pp[:],
                                         lhsT=v_pair[j][:, :, dd * 128:(dd + 1) * 128],
                                         rhs=pt_pair[j][:],
                                         start=(j == 0), stop=(j == NKP - 1),
                                         perf_mode=DR)
                    at = sb.tile([128, QC], F32, name="at", tag="at", bufs=2)
                    nc.vector.tensor_mul(out=at[:], in0=pp[:], in1=recip[:])
                    t = sb.tile([128, QC], BF16, name=f"qa{dd}", tag=f"qa{dd}", bufs=2)
                    nc.vector.tensor_add(out=t[:], in0=at[:], in1=qt_f32[dd][:])
                    qa_bf.append(t)
                    if qc + 1 < NQC:
                        for kt in range(dd * 8, dd * 8 + 8):
                            emit_s_tile(kt, qt_cur[1], pt_cur)
                # O = (Q + A) @ Wo^T + bo2
                for rb in range(QC // 128):
                    pp = ps.tile([128, C], F32, name="ops", tag="pB", bufs=3)
                    for cc in range(CCH):
                        nc.tensor.matmul(pp[:], lhsT=qa_bf[cc][:, rb * 128:(rb + 1) * 128],
                                         rhs=wo_b[cc][:],
                                         start=(cc == 0), stop=(cc == CCH - 1))
                    ot = sb.tile([128, C], F32, name="ot", tag="ot", bufs=3)
                    nc.vector.tensor_add(out=ot[:], in0=pp[:], in1=bo_bc[:])
                    nc.sync.dma_start(out=out_d[q0 + rb * 128:q0 + (rb + 1) * 128, :],
                                      in_=ot[:])

    nc.compile()
    return nc


def get_built():
    global _BUILT
    if _BUILT is None:
        _BUILT = build()
    return _BUILT


def make_in_maps(x1, x2, Wq, bq, Wk, bk, Wv, bv, Wo, bo):
    bf = ml_dtypes.bfloat16
    f8 = ml_dtypes.float8_e4m3
    wq_t = np.ascontiguousarray(Wq.T).astype(bf)
    wk_t = np.ascontiguousarray(Wk.T).astype(f8)
    wv_t = np.ascontiguousarray(Wv.T).astype(f8)
    wo_t = np.ascontiguousarray(Wo.T).astype(bf)
    bo2 = (Wo @ bv + bo).astype(np.float32)
    x2t_b = [np.ascontiguousarray(x2[b].T).astype(f8) for b in range(B)]
    in_maps = []
    for cid in range(NCORES):
        b, h = cid // 2, cid % 2
        x1t = np.ascontiguousarray(x1[b, h * QROWS:(h + 1) * QROWS, :].T).astype(bf)
        in_maps.append({
            "x1t": x1t,
            "x2t": x2t_b[b],
            "wq_t": wq_t, "wk_t": wk_t, "wv_t": wv_t, "wo_t": wo_t,
            "bq": bq.astype(np.float32), "bo2": bo2,
        })
    return in_maps


LAST_RESULT = None


def kernel(x1, x2, Wq, bq, Wk, bk, Wv, bv, Wo, bo):
    global LAST_RESULT
    nc = get_built()
    in_maps = make_in_maps(x1, x2, Wq, bq, Wk, bk, Wv, bv, Wo, bo)
    trace = bool(os.environ.get("KERNEL_TRACE"))
    res = run_bass_kernel_spmd(nc, in_maps, core_ids=list(range(NCORES)), trace=trace)
    LAST_RESULT = res
    out = np.empty((B, N1, C), dtype=np.float32)
    for cid in range(NCORES):
        b, h = cid // 2, cid % 2
        out[b, h * QROWS:(h + 1) * QROWS, :] = res.results[cid]["out"]
    return out
